# revision 1
# baseline (speedup 1.0000x reference)
"""KMeans assignment (VQ codebook) kernel for 8 Trainium2 NeuronCores.

Problem: x [4, 3, 256, 256] f32, C [512, 3] f32 ->
         argmin_k ||x_pixel - C_k||^2 as int32 [4, 65536].

Strategy (data-parallel over pixels, codebook replicated):
  - 262144 pixels split as 32768 per core (core c: batch c//2, half c%2).
  - Per core, per 128-pixel tile: PE matmul computes
        s[pix, k] = 2*x.C_k - ||C_k||^2   (K=4 contraction: 3 channels + ones
        row against bias row), so argmax_k s == argmin_k d, exact fp32.
  - ScalarE copies PSUM->SBUF; VectorE max + max_index give the argmax index
    (first occurrence on ties, matching jnp.argmin tie-break).
  - Indices collected in SBUF, one DMA out; host re-lays [128, 256] -> pixels.
"""

import numpy as np

import concourse.bass as bass
import concourse.tile as tile
from concourse import bacc, mybir
from concourse.bass_utils import run_bass_kernel_spmd

NCORES = 8
NPIX = 32768  # pixels per core
PTILE = 128  # pixels per matmul tile
NTILES = NPIX // PTILE  # 256
NCLUS = 512
NCH = 3

_CACHE = {}


def build_nc():
    """Build + compile the single-core SPMD program (same NEFF on all cores)."""
    nc = bacc.Bacc(
        "TRN2", target_bir_lowering=False, debug=False, num_devices=NCORES
    )
    xs = nc.dram_tensor("xs", [4, NPIX], mybir.dt.float32, kind="ExternalInput").ap()
    ct = nc.dram_tensor("ct", [4, NCLUS], mybir.dt.float32, kind="ExternalInput").ap()
    out = nc.dram_tensor(
        "out", [128, NTILES * 8], mybir.dt.uint32, kind="ExternalOutput"
    ).ap()

    with tile.TileContext(nc) as tc:
        with (
            tc.tile_pool(name="const", bufs=1) as const_pool,
            tc.tile_pool(name="psum", bufs=8, space="PSUM") as psum_pool,
            tc.tile_pool(name="sb", bufs=6) as sb_pool,
            tc.tile_pool(name="mx", bufs=6) as mx_pool,
        ):
            xe = const_pool.tile([4, NPIX], mybir.dt.float32)
            ctt = const_pool.tile([4, NCLUS], mybir.dt.float32)
            ob = const_pool.tile([128, NTILES * 8], mybir.dt.uint32)

            nc.sync.dma_start(ctt[:], ct[:])
            # chunked input load so early tiles can start before the full
            # 512KB shard lands
            nchunk = 8
            csz = NPIX // nchunk
            for i in range(nchunk):
                nc.sync.dma_start(
                    xe[:, bass.ts(i, csz)], xs[:, bass.ts(i, csz)]
                )

            for t in range(NTILES):
                ps = psum_pool.tile([128, NCLUS], mybir.dt.float32)
                nc.tensor.matmul(
                    ps[:],
                    lhsT=xe[:, bass.ts(t, PTILE)],
                    rhs=ctt[:],
                    start=True,
                    stop=True,
                )
                sb = sb_pool.tile([128, NCLUS], mybir.dt.float32)
                nc.scalar.copy(sb[:], ps[:])
                mx = mx_pool.tile([128, 8], mybir.dt.float32)
                nc.vector.max(mx[:], sb[:])
                nc.vector.max_index(ob[:, bass.ts(t, 8)], mx[:], sb[:])

            nc.sync.dma_start(out[:], ob[:])

    nc.compile()
    return nc


def get_nc():
    if "nc" not in _CACHE:
        _CACHE["nc"] = build_nc()
    return _CACHE["nc"]


def make_in_maps(x, C):
    """Host-side shard prep: slice pixels per core, replicate codebook rows."""
    x = np.asarray(x, dtype=np.float32)
    C = np.asarray(C, dtype=np.float32)
    bs, c, h, w = x.shape
    xf = np.ascontiguousarray(x.reshape(bs, c, h * w))

    ctm = np.empty((4, NCLUS), dtype=np.float32)
    ctm[0:3] = 2.0 * C.T
    ctm[3] = -(C * C).sum(axis=1)

    in_maps = []
    for core in range(NCORES):
        b, half = divmod(core, 2)
        sh = np.empty((4, NPIX), dtype=np.float32)
        sh[0:3] = xf[b, :, half * NPIX : (half + 1) * NPIX]
        sh[3] = 1.0
        in_maps.append({"xs": sh, "ct": ctm})
    return in_maps


def gather_out(results, bs, hw_count):
    out = np.empty((bs, hw_count), dtype=np.int32)
    for core in range(NCORES):
        b, half = divmod(core, 2)
        ob = results[core]["out"].reshape(128, NTILES, 8)[:, :, 0]
        # ob[p, t] = index of pixel t*128 + p within this core's shard
        out[b, half * NPIX : (half + 1) * NPIX] = ob.T.reshape(-1).astype(np.int32)
    return out


def kernel(x, C):
    x = np.asarray(x)
    bs, c, h, w = x.shape
    in_maps = make_in_maps(x, C)
    nc = get_nc()
    res = run_bass_kernel_spmd(nc, in_maps, list(range(NCORES))).results
    return gather_out(res, bs, h * w)
